# revision 47
# baseline (speedup 1.0000x reference)
"""Longformer attention TP-sharded Bass kernel for 8 NeuronCores (v4).

Sharding: tensor-parallel over heads. Core d owns heads 2d, 2d+1:
  - Wq/Wk/Wv rows [128d:128(d+1)]  (nn.Linear: q = x @ Wq.T)
  - Wo columns [128d:128(d+1)]
  Each core computes its heads' sparse (windowed+global) attention and a
  full-size out-proj partial; host sums the 8 partials (the "all-reduce").

v4 structure (bf16 compute, fp32 PSUM):
  - Inputs host-packed so every DMA moves >=2KB per descriptor.
  - Projections (per 512-token chunk) interleave with the attention loop:
    query block qb is emitted as soon as its k/q/v chunks are projected,
    so PE-dense projection streams fill the attention loop's latency gaps.
  - scoresT [k, q] per qb in ONE psum tile [128, 896]:
    [A-lo|A-diag|A-up | B-lo|B-diag|B-up | key0-strips] where the strips
    are [1, 128] key-0 score rows (head A at partition 0, B at 32).
    One exp covers the tile; gpsimd applies the band masks (off the
    critical path thanks to the 2-stage software pipeline).
  - global query row (q=0) via N=1 rider matmuls on the diag stationaries
    into a persistent [128, 64] psum tile; processed in a small tail,
    with qb 0's normalize/out-proj deferred to the end.
"""

import os
import numpy as np
import ml_dtypes

S = 4096
HIDDEN = 1024
N_CORES = 8
OC = 128          # out-proj contraction dims (head dims) per core = 2 heads x 64
NQB = S // 128    # 32 query/key blocks
BF16 = ml_dtypes.bfloat16

_CACHE = {}
LAST_RESULTS = None


def _masks_np():
    """Triangle masks only (the diagonal block is all-ones and unmasked),
    2-head-concatenated planes [128, 4*256]: lo, up, lo&(k>0), up|(q==0)."""
    p = np.arange(128)[:, None]   # key index within block
    f = np.arange(128)[None, :]   # query index within block
    lo = (f <= p)
    up = (f >= p)
    up0 = up | (f == 0)
    lo_nok0 = lo & (p > 0)
    out = np.stack([np.tile(m, (1, 2)) for m in (lo, up, lo_nok0, up0)])
    return np.ascontiguousarray(out.transpose(1, 0, 2).reshape(128, 4 * 256)
                                ).astype(BF16)


def _qb_plan(qb):
    """(key blocks per head, width per head, strip col or None, diag_j,
    mask ops as [(plane, col block j)])"""
    if qb == 0:
        return [0, 1], 256, None, 0, [(3, 1)]
    if qb == 1:
        return [0, 1, 2], 384, 768, 1, [(2, 0), (1, 2)]
    if qb == NQB - 1:
        return [NQB - 2, NQB - 1], 256, 512, 1, [(0, 0)]
    return [qb - 1, qb, qb + 1], 384, 768, 1, [(0, 0), (1, 2)]


def _build():
    import concourse.bass as bass
    import concourse.mybir as mybir
    import concourse.tile as tile
    from concourse import bacc

    f32 = mybir.dt.float32
    bf16 = mybir.dt.bfloat16
    Exp = mybir.ActivationFunctionType.Exp

    nc = bacc.Bacc("TRN2", target_bir_lowering=False, debug=False,
                   num_devices=N_CORES)

    # host-packed layouts: inner dims contiguous per partition for fat DMAs
    xt_d = nc.dram_tensor("xtp", [128, 8, 8, 512], bf16,
                          kind="ExternalInput").ap()   # [p, sc, hc, t]
    wq_d = nc.dram_tensor("wqp", [128, 8, OC], bf16, kind="ExternalInput").ap()
    wk_d = nc.dram_tensor("wkp", [128, 8, OC], bf16, kind="ExternalInput").ap()
    wv_d = nc.dram_tensor("wvp", [128, 8, OC], bf16, kind="ExternalInput").ap()
    wo_d = nc.dram_tensor("wot", [OC, HIDDEN], bf16, kind="ExternalInput").ap()
    out_d = nc.dram_tensor("partial", [S, HIDDEN], bf16,
                           kind="ExternalOutput").ap()
    mask_d = nc.inline_tensor(_masks_np(), name="masks").ap()
    id_d = nc.inline_tensor(np.eye(128, dtype=BF16), name="ident").ap()

    with tile.TileContext(nc) as tc:
        import contextlib
        with contextlib.ExitStack() as ctx:
            big = ctx.enter_context(tc.tile_pool(name="big", bufs=1))
            tmp = ctx.enter_context(tc.tile_pool(name="tmp", bufs=3))
            pbig = ctx.enter_context(tc.tile_pool(name="pbig", bufs=2,
                                                  space="PSUM"))
            p512 = ctx.enter_context(tc.tile_pool(name="p512", bufs=2,
                                                  space="PSUM"))
            pper = ctx.enter_context(tc.tile_pool(name="pper", bufs=1,
                                                  space="PSUM"))

            # ---- resident tensors ----
            xt_sb = big.tile([128, 8, 8, 512], bf16)  # [p, sc, hc, t]
            qt_sb = big.tile([128, S], bf16)          # q.T * 0.125
            kt_sb = big.tile([128, S], bf16)
            v_sb = big.tile([128, NQB, 130], bf16)    # [vA|1|vB|1] per key block
            v0bc = big.tile([128, 130], bf16)         # v[key0] bcast to all parts
            outn_sb = big.tile([128, NQB, 128], bf16)  # attn out, natural [q, hd]
            outt_sb = big.tile([128, NQB, 128], bf16)  # transposed [hd, q]
            p0col_sb = big.tile([128, 64], bf16)       # q0-row probs (col=2qb+h)
            wq_sb = big.tile([128, 8, OC], bf16)
            wk_sb = big.tile([128, 8, OC], bf16)
            wv_sb = big.tile([128, 8, OC], bf16)
            wo_sb = big.tile([128, HIDDEN], bf16)
            mask_sb = big.tile([128, 4, 256], bf16)
            id_sb = big.tile([128, 128], bf16)

            # PSUM slots are bank (2KB) granular: pack small accumulators into
            # two manually-subdivided banks.
            # bankA: q0col [0:64], pso0 [64:194], pstr [256:320],[320:384]
            bankA = pper.tile([128, 512], f32, name="bankA")
            # bankB: pso even [0:130], pso odd [192:322], pq0 row [352:482]
            bankB = pper.tile([128, 512], f32, name="bankB")
            q0col = bankA[:, 0:64]
            pso0 = bankA[:, 64:194]
            pstr_slots = [bankA[:, 256:320].bitcast(bf16),
                          bankA[:, 320:384].bitcast(bf16)]
            pso_slots = [bankB[:, 0:130], bankB[:, 192:322]]
            pq0 = bankB[0:1, 352:482]
            scratch = bankA[:, 384:512]   # filler-matmul target, never read

            NFILL = int(os.environ.get("KERNEL_FILL", "0"))

            def emit_fill(n):
                # dep-free weight loads that keep the PE activity monitor
                # "busy" through short stalls so the HAM clock stays at 2.4GHz
                for _ in range(n):
                    nc.tensor.ldweights(weights=wo_sb[:, 0:128])

            # ---- weight/constant loads (fat descriptors, earliest first) ----
            nc.sync.dma_start(wq_sb, wq_d)
            nc.sync.dma_start(wv_sb, wv_d)
            nc.sync.dma_start(wk_sb, wk_d)
            nc.sync.dma_start(xt_sb[:, 0], xt_d[:, 0])
            nc.sync.dma_start(wo_sb, wo_d)
            nc.sync.dma_start(mask_sb,
                              mask_d.rearrange("p (m f) -> p m f", m=4))
            nc.sync.dma_start(id_sb, id_d)
            nc.vector.memset(v_sb[:, :, 64], 1.0)
            nc.vector.memset(v_sb[:, :, 129], 1.0)
            nc.vector.memset(q0col, 0.0)

            # ---- emission helpers ----
            probs_t = {}

            def emit_proj(sc):
                if sc + 1 < 8:   # prefetch next x chunk
                    nc.sync.dma_start(xt_sb[:, sc + 1], xt_d[:, sc + 1])
                ssl = slice(sc * 512, (sc + 1) * 512)
                psv = p512.tile([128, 512], f32, tag="ps512", name="psv")
                psq = p512.tile([128, 512], f32, tag="ps512", name="psq")
                psk = p512.tile([128, 512], f32, tag="ps512", name="psk")
                # interleave v (stationary=xt, N=128) with q/k (N=512) so the
                # frequent v LDWEIGHTS hide under long q/k streams
                for b in range(2):
                    bsl = slice(b * 128, b * 128 + 128)
                    for hc in range(8):
                        nc.tensor.matmul(psv[:, bsl],
                                         xt_sb[:, sc, hc, bsl],
                                         wv_sb[:, hc, :],
                                         start=(hc == 0), stop=(hc == 7))
                for hc in range(8):
                    nc.tensor.matmul(psq, wq_sb[:, hc, :], xt_sb[:, sc, hc, :],
                                     start=(hc == 0), stop=(hc == 7))
                for b in range(2, 4):
                    bsl = slice(b * 128, b * 128 + 128)
                    for hc in range(8):
                        nc.tensor.matmul(psv[:, bsl],
                                         xt_sb[:, sc, hc, bsl],
                                         wv_sb[:, hc, :],
                                         start=(hc == 0), stop=(hc == 7))
                for hc in range(8):
                    nc.tensor.matmul(psk, wk_sb[:, hc, :], xt_sb[:, sc, hc, :],
                                     start=(hc == 0), stop=(hc == 7))

                # fold the 1/sqrt(hd) = 0.125 softmax scale into q
                nc.vector.tensor_scalar_mul(qt_sb[:, ssl], psq, 0.125)
                nc.scalar.copy(kt_sb[:, ssl], psk)
                # v: [4 blocks][A64|B64] -> v_sb [kb][A|1|B|1] in one copy
                vdst = v_sb[:, sc * 4:sc * 4 + 4, :].rearrange(
                    "p b (h c) -> p b h c", h=2)
                nc.vector.tensor_copy(
                    vdst[:, :, :, 0:64],
                    psv.rearrange("p (b h c) -> p b h c", b=4, h=2))
                if sc == 0:
                    # v[key0] broadcast to all partitions (rank-1 key0 PV)
                    nc.gpsimd.partition_broadcast(v0bc, v_sb[0:1, 0, :])

            def emit_scores(qb):
                qsl = slice(qb * 128, (qb + 1) * 128)
                kbs, whead, strip, diag_j, mops = _qb_plan(qb)
                W = (strip + 128) if strip is not None else 2 * whead

                pss = pbig.tile([128, 896], f32, tag="pss", name="pss")
                for h in range(2):
                    bp = 64 * h
                    for j, kb in enumerate(kbs):
                        off = h * whead + j * 128
                        nc.tensor.matmul(
                            pss[:, off:off + 128],
                            kt_sb[bp:bp + 64, kb * 128:(kb + 1) * 128],
                            qt_sb[bp:bp + 64, qsl],
                            start=True, stop=True)
                        if kb == qb and qb >= 2:
                            # rider: q=0 scores vs this key block
                            nc.tensor.matmul(
                                q0col[:, 2 * qb + h:2 * qb + h + 1],
                                kt_sb[bp:bp + 64, kb * 128:(kb + 1) * 128],
                                qt_sb[bp:bp + 64, 0:1],
                                start=True, stop=True)
                    if strip is not None:
                        # key-0 scores [1, 128] at partition 0 (A) / 32 (B)
                        sp = 32 * h
                        nc.tensor.matmul(
                            pss[sp:sp + 1, strip:strip + 128],
                            kt_sb[bp:bp + 64, 0:1],
                            qt_sb[bp:bp + 64, qsl],
                            start=True, stop=True)

                probs = tmp.tile([128, 896], bf16, tag="probs", name="probs")
                probs_t[qb] = probs
                nc.scalar.activation(probs[:, :W], pss[:, :W], Exp)
                # mask only the triangular blocks; diag/strips stay exp-only
                probs_h = probs[:, :2 * whead].rearrange("p (h w) -> p h w",
                                                         h=2)
                for plane, j in mops:
                    psl = probs_h[:, :, j * 128:(j + 1) * 128]
                    nc.gpsimd.tensor_mul(
                        psl, psl,
                        mask_sb[:, plane, :].rearrange("p (h c) -> p h c",
                                                       h=2))

            def emit_pv(qb):
                kbs, whead, strip, diag_j, mops = _qb_plan(qb)
                probs = probs_t.pop(qb)
                pso_t = pso0 if qb == 0 else pso_slots[qb % 2]
                for h in range(2):
                    hs = slice(65 * h, 65 * h + 65)
                    # diag + key0 rank-1 first: they wait only on exp, not on
                    # the gpsimd triangle masks
                    order = [diag_j] + [j for j in range(len(kbs))
                                        if j != diag_j]
                    nmm = len(order) + (1 if strip is not None else 0)
                    mmi = 0
                    for j in order:
                        off = h * whead + j * 128
                        nc.tensor.matmul(
                            pso_t[:, hs], probs[:, off:off + 128],
                            v_sb[:, kbs[j], hs],
                            start=(mmi == 0), stop=(mmi == nmm - 1),
                            skip_group_check=True)
                        mmi += 1
                        if mmi == 1 and strip is not None:
                            sp = 32 * h
                            nc.tensor.matmul(
                                pso_t[:, hs],
                                probs[sp:sp + 1, strip:strip + 128],
                                v0bc[sp:sp + 1, hs],
                                start=False, stop=False,
                                skip_group_check=True)
                            mmi += 1

            def emit_norm(qb):
                """pso [q, A|dA|B|dB] -> outn = numerators / denominators"""
                pso_t = pso0 if qb == 0 else pso_slots[qb % 2]
                pso_h = pso_t.rearrange("p (h c) -> p h c", h=2)
                recip = tmp.tile([128, 2], f32, tag="recip", name="recip")
                nc.vector.reciprocal(recip, pso_h[:, :, 64])
                for h in range(2):
                    nc.vector.tensor_scalar_mul(
                        outn_sb[:, qb, 64 * h:64 * h + 64],
                        pso_h[:, h, 0:64], recip[:, h:h + 1])

            def emit_proj_out(qb):
                """outn -> transpose -> @Wo -> bf16 stage -> DMA (phase C)"""
                pstr = pstr_slots[qb % 2]
                nc.tensor.transpose(pstr, outn_sb[:, qb, :], id_sb)
                nc.vector.tensor_copy(outt_sb[:, qb, :], pstr)
                stage = tmp.tile([128, HIDDEN], bf16, tag="stage", name="stage")
                for oc in range(2):
                    psp = p512.tile([128, 512], f32, tag="ps512", name="psp")
                    nc.tensor.matmul(psp, outt_sb[:, qb, :],
                                     wo_sb[:, oc * 512:(oc + 1) * 512],
                                     start=True, stop=True)
                    osl = slice(oc * 512, (oc + 1) * 512)
                    if oc == 0:
                        nc.vector.tensor_copy(stage[:, osl], psp)
                    else:
                        nc.scalar.copy(stage[:, osl], psp)
                    nc.sync.dma_start(out_d[qb * 128:(qb + 1) * 128, osl],
                                      stage[:, osl])

            # ---- fused phase A + phase B (2-stage pipelined) emission ----
            state = {"s": 0, "p": 0}

            def advance(limit):
                while state["s"] <= min(limit, NQB - 1):
                    emit_scores(state["s"])
                    state["s"] += 1
                    while state["p"] < state["s"] - 1:
                        qb = state["p"]
                        emit_pv(qb)
                        if qb >= 1:
                            emit_norm(qb)
                            if not PHASE_C:
                                emit_proj_out(qb)
                        state["p"] += 1

            INTERLEAVE = int(os.environ.get("KERNEL_INTERLEAVE", "0"))
            PHASE_C = int(os.environ.get("KERNEL_PHASE_C", "0"))
            for sc in range(8):
                emit_proj(sc)
                if INTERLEAVE:
                    advance(4 * sc + 2)
            advance(NQB - 1)
            while state["p"] < NQB:
                qb = state["p"]
                emit_pv(qb)
                if qb >= 1:
                    emit_norm(qb)
                    if not PHASE_C:
                        emit_proj_out(qb)
                state["p"] += 1

            # ---- tail: q0 row (keys 256+) and deferred qb 0 ----
            nc.scalar.activation(p0col_sb, q0col, Exp)
            for h in range(2):
                hs = slice(65 * h, 65 * h + 65)
                for kb in range(2, NQB):
                    nc.tensor.matmul(
                        pq0[:, hs], p0col_sb[:, 2 * kb + h:2 * kb + h + 1],
                        v_sb[:, kb, hs],
                        start=(kb == 2), stop=(kb == NQB - 1),
                        skip_group_check=True)
            # patch q0 row (numerator + denominator) into qb0's accumulator
            pq0_sb = tmp.tile([1, 130], f32, tag="pq0sb", name="pq0_sb")
            nc.vector.tensor_copy(pq0_sb, pq0)
            nc.vector.tensor_add(pso0[0:1, :], pso0[0:1, :], pq0_sb)
            emit_norm(0)
            if PHASE_C:
                # dense out-proj sweep at the end (warm-clock friendly)
                for qb in range(NQB):
                    emit_proj_out(qb)
            else:
                emit_proj_out(0)

    nc.compile()
    return nc


def kernel(x, Wq, Wk, Wv, Wo):
    from concourse import bass_utils

    x = np.asarray(x)
    B = x.shape[0]
    # [p, sc, hc, t] packing: x.T row h = hc*128+p, col s = sc*512+t
    xtp = np.ascontiguousarray(
        np.asarray(x)[0].T.astype(BF16).reshape(8, 128, 8, 512)
        .transpose(1, 2, 0, 3))

    def packw(w):   # rows of W.T: h = hc*128+p -> [p, hc, o]
        return np.ascontiguousarray(
            w.T.astype(BF16).reshape(8, 128, OC).transpose(1, 0, 2))

    in_maps = []
    for d in range(N_CORES):
        rs = slice(OC * d, OC * (d + 1))
        in_maps.append({
            "xtp": xtp,
            "wqp": packw(np.asarray(Wq)[rs, :]),
            "wkp": packw(np.asarray(Wk)[rs, :]),
            "wvp": packw(np.asarray(Wv)[rs, :]),
            "wot": np.ascontiguousarray(np.asarray(Wo)[:, rs].T.astype(BF16)),
        })

    if "nc" not in _CACHE:
        _CACHE["nc"] = _build()
    nc = _CACHE["nc"]

    res = bass_utils.run_bass_kernel_spmd(
        nc, in_maps, core_ids=list(range(N_CORES)),
        trace=bool(os.environ.get("KERNEL_TRACE")))
    global LAST_RESULTS
    LAST_RESULTS = res

    out = np.zeros((S, HIDDEN), np.float64)
    for r in res.results:
        out += r["partial"].astype(np.float64)
    return out.reshape(B, S, HIDDEN).astype(np.float32)
